# revision 4
# baseline (speedup 1.0000x reference)
"""Local-strided block-sparse paged attention (decode) on 8 Trainium2 cores.

Sharding: batch across cores (2 sequences/core, all 16 q-heads).
Host (numpy) resolves the CSR layout + block_tables into per-(b,h) gathered,
transposed K/V panels in bf16; each core streams them from HBM in 2MB
batched DMAs and runs QK -> softmax -> PV on PE/ACT/DVE. Outputs are the
unnormalized PV accumulator plus the per-partition exp-sum; the final
(tiny) normalization happens on host.
"""
import numpy as np

B, H, KVH, D, X = 16, 16, 4, 128, 4
BLK, MAXB = 16, 256
J = 64                      # max kv blocks per row (CSR rows are never longer)
N = J * BLK                 # 1024 padded tokens per (b,h) row
NCH = N // 128              # 128-token chunks per row
NC_CORES = 8
SEQ_PER_CORE = B // NC_CORES
ROWS = SEQ_PER_CORE * H     # rows handled per core
RPG = 4                     # rows per DMA group (2MB bf16 per transfer)
NG = ROWS // RPG
GW = RPG * 2 * N            # bf16 elems per partition line per group
SM_SCALE = 1.0 / float(np.sqrt(D))


def _build_device_program():
    import concourse.bacc as bacc
    import concourse.mybir as mybir
    from concourse.tile import TileContext

    f32 = mybir.dt.float32
    bf16 = mybir.dt.bfloat16
    nc = bacc.Bacc("TRN2", target_bir_lowering=False)
    kv = nc.dram_tensor("kv", [NG, 128, GW], bf16, kind="ExternalInput")
    mt = nc.dram_tensor("mt", [128, ROWS * NCH], f32, kind="ExternalInput")
    qq = nc.dram_tensor("qq", [128, ROWS], bf16, kind="ExternalInput")
    out = nc.dram_tensor("out", [128, 2 * ROWS], f32, kind="ExternalOutput")

    with TileContext(nc) as tc:
        with (
            tc.tile_pool(name="kv", bufs=3) as kvp,
            tc.tile_pool(name="small", bufs=4) as sp,
            tc.tile_pool(name="ps_sc", bufs=3, space="PSUM") as pp_sc,
            tc.tile_pool(name="ps_ov", bufs=3, space="PSUM") as pp_ov,
            tc.tile_pool(name="io", bufs=1) as iop,
        ):
            mtt = iop.tile([128, ROWS * NCH], f32, tag="mt")
            nc.sync.dma_start(out=mtt[:], in_=mt[:, :])
            qt = iop.tile([128, ROWS], bf16, tag="qt")
            nc.sync.dma_start(out=qt[:], in_=qq[:, :])
            osb = iop.tile([128, 2 * ROWS], f32, tag="osb")

            kvt = None
            for r in range(ROWS):
                g, m = divmod(r, RPG)
                if m == 0:
                    kvt = kvp.tile([128, GW], bf16, tag="kv")
                    nc.sync.dma_start(out=kvt[:], in_=kv[g])
                base = m * 2 * N

                # scores_T[t, c] = sum_d K[d, c*128+t] * q[d]
                sc = pp_sc.tile([128, NCH], f32, tag="sc")
                for c in range(NCH):
                    nc.tensor.matmul(
                        sc[:, c:c + 1],
                        kvt[:, base + 128 * c:base + 128 * (c + 1)],
                        qt[:, r:r + 1],
                        start=True, stop=True,
                    )
                ssb = sp.tile([128, NCH], f32, tag="ssb")
                nc.vector.tensor_add(ssb[:], sc[:], mtt[:, r * NCH:(r + 1) * NCH])
                p = sp.tile([128, NCH], bf16, tag="p")
                nc.scalar.activation(
                    p[:], ssb[:], mybir.ActivationFunctionType.Exp,
                    scale=SM_SCALE, accum_out=osb[:, ROWS + r:ROWS + r + 1],
                )

                # out[d] = sum_t P_T[t] * V_T[t, d], V chunk stationary
                ov = pp_ov.tile([128, 1], f32, tag="ov")
                vbase = base + N
                for c in range(NCH):
                    nc.tensor.matmul(
                        ov[:],
                        kvt[:, vbase + 128 * c:vbase + 128 * (c + 1)],
                        p[:, c:c + 1],
                        start=(c == 0), stop=(c == NCH - 1),
                    )
                nc.vector.tensor_copy(osb[:, r:r + 1], ov[:])

            nc.sync.dma_start(out=out[:, :], in_=osb[:])
    nc.compile()
    return nc


_NC_CACHE = None
_LAST_RES = None
_LAST_IN_MAPS = None


def kernel(q, k_cache, v_cache, block_tables, context_lens, layout_crow, layout_col):
    import ml_dtypes
    from concourse.bass_utils import run_bass_kernel_spmd

    bf16 = ml_dtypes.bfloat16
    q = np.asarray(q, np.float32)
    k_cache = np.asarray(k_cache, np.float32)
    v_cache = np.asarray(v_cache, np.float32)
    block_tables = np.asarray(block_tables, np.int32)
    context_lens = np.asarray(context_lens, np.int32)
    layout_crow = np.asarray(layout_crow, np.int32)
    layout_col = np.asarray(layout_col, np.int32)

    # ---- host: resolve CSR rows into gathered K/V panels (mirrors reference) ----
    q_pid = context_lens.astype(np.int64) - 1            # [B]
    pbid = q_pid // BLK
    h_idx = np.arange(H)
    hkv = h_idx // (H // KVH)
    start = layout_crow[h_idx[None, :], pbid[:, None]]   # [B,H]
    end = layout_crow[h_idx[None, :], pbid[:, None] + 1]
    jj = np.arange(J)
    idx = start[:, :, None] + jj                         # [B,H,J]
    valid = idx < end[:, :, None]
    idx = np.clip(idx, 0, layout_col.shape[1] - 1)
    cols = layout_col[h_idx[None, :, None], idx]         # [B,H,J]
    bt = block_tables[np.arange(B)[:, None, None], cols]  # [B,H,J]

    kcb = k_cache.astype(bf16)
    vcb = v_cache.astype(bf16)
    kb = kcb[bt, hkv[None, :, None]]                     # [B,H,J,32,16,4]
    # K d-major: Kd[b,h,d,(j,n)] with d = dx*4+xi
    kd_full = np.ascontiguousarray(
        kb.transpose(0, 1, 3, 5, 2, 4).reshape(B, H, 128, N)
    )
    vb = vcb[bt, hkv[None, :, None]]                     # [B,H,J,128,16]
    # V token-major, chunk-interleaved: Vt[b,h,t,(c,d)] = V[d, c*128+t]
    v_t = vb.transpose(0, 1, 2, 4, 3).reshape(B, H, N, 128)          # [(j,n), d]
    vt_full = np.ascontiguousarray(
        v_t.reshape(B, H, NCH, 128, 128).transpose(0, 1, 3, 2, 4)
        .reshape(B, H, 128, N)
    )
    pos = cols[..., None] * BLK + np.arange(BLK)          # [B,H,J,BLK]
    mask = valid[..., None] & (pos <= q_pid[:, None, None, None])
    mask = mask.reshape(B, H, N)
    madd = np.where(mask, np.float32(0.0), np.float32(-1e9))
    # mask_T[t, r*NCH + c] = madd[r, c*128 + t]
    qbf = q.astype(bf16)

    global _NC_CACHE, _LAST_RES
    if _NC_CACHE is None:
        _NC_CACHE = _build_device_program()
    nc = _NC_CACHE

    in_maps = []
    for core in range(NC_CORES):
        bs = slice(core * SEQ_PER_CORE, (core + 1) * SEQ_PER_CORE)
        kd_c = kd_full[bs].reshape(ROWS, 128, N)
        vt_c = vt_full[bs].reshape(ROWS, 128, N)
        kvg = np.empty((NG, 128, RPG, 2, N), bf16)
        kvg[:, :, :, 0, :] = kd_c.reshape(NG, RPG, 128, N).transpose(0, 2, 1, 3)
        kvg[:, :, :, 1, :] = vt_c.reshape(NG, RPG, 128, N).transpose(0, 2, 1, 3)
        mt_c = np.ascontiguousarray(
            madd[bs].reshape(ROWS, NCH, 128).transpose(2, 0, 1)
            .reshape(128, ROWS * NCH)
        )
        q_c = np.ascontiguousarray(qbf[bs].reshape(ROWS, 128).T)
        in_maps.append({
            "kv": kvg.reshape(NG, 128, GW),
            "mt": mt_c,
            "qq": q_c,
        })

    global _LAST_IN_MAPS
    _LAST_IN_MAPS = in_maps
    res = run_bass_kernel_spmd(nc, in_maps, core_ids=list(range(NC_CORES)))
    _LAST_RES = res
    out = np.empty((B, H, D), np.float32)
    for core in range(NC_CORES):
        o = res.results[core]["out"]                     # [128, 2*ROWS] fp32
        ov = o[:, :ROWS]
        denom = o[:, ROWS:].sum(axis=0)                  # [ROWS]
        bs = slice(core * SEQ_PER_CORE, (core + 1) * SEQ_PER_CORE)
        out[bs] = (ov / denom[None, :]).T.reshape(SEQ_PER_CORE, H, D)
    return out


# revision 6
# speedup vs baseline: 64.4373x; 64.4373x over previous
"""Local-strided block-sparse paged attention (decode) on 8 Trainium2 cores.

Sharding: batch across cores (2 sequences/core, all 16 q-heads).
Host (numpy) resolves the CSR layout + block_tables into per-(b,h) gathered,
transposed K/V panels in bf16; each core streams them from HBM in 2MB
batched DMAs and runs QK -> softmax -> PV on PE/ACT/DVE. Outputs are the
unnormalized PV accumulator plus the per-partition exp-sum; the final
(tiny) normalization happens on host.
"""
import numpy as np

B, H, KVH, D, X = 16, 16, 4, 128, 4
BLK, MAXB = 16, 256
J = 64                      # max kv blocks per row (CSR rows are never longer)
N = J * BLK                 # 1024 padded tokens per (b,h) row
NCH = N // 128              # 128-token chunks per row
NC_CORES = 8
SEQ_PER_CORE = B // NC_CORES
ROWS = SEQ_PER_CORE * H     # rows handled per core
RPG = 4                     # rows per DMA group (2MB bf16 per transfer)
NG = ROWS // RPG
GW = RPG * 2 * N            # bf16 elems per partition line per group
SM_SCALE = 1.0 / float(np.sqrt(D))


def _build_device_program(reps=1):
    import concourse.bacc as bacc
    import concourse.mybir as mybir
    from concourse.tile import TileContext
    import contextlib

    f32 = mybir.dt.float32
    bf16 = mybir.dt.bfloat16
    nc = bacc.Bacc("TRN2", target_bir_lowering=False)
    kv = nc.dram_tensor("kv", [NG, 128, GW], bf16, kind="ExternalInput")
    mt = nc.dram_tensor("mt", [128, ROWS * NCH], f32, kind="ExternalInput")
    qq = nc.dram_tensor("qq", [128, ROWS], bf16, kind="ExternalInput")
    out = nc.dram_tensor("out", [128, 2 * ROWS], f32, kind="ExternalOutput")

    with TileContext(nc) as tc:
        with (
            tc.tile_pool(name="kv", bufs=3) as kvp,
            tc.tile_pool(name="small", bufs=4) as sp,
            tc.tile_pool(name="ps_sc", bufs=3, space="PSUM") as pp_sc,
            tc.tile_pool(name="ps_ov", bufs=3, space="PSUM") as pp_ov,
            tc.tile_pool(name="io", bufs=1) as iop,
        ):
            rep_ctx = tc.For_i(0, reps, 1) if reps > 1 else contextlib.nullcontext()
            with rep_ctx:
                mtt = iop.tile([128, ROWS * NCH], f32, tag="mt")
                nc.sync.dma_start(out=mtt[:], in_=mt[:, :])
                qt = iop.tile([128, ROWS], bf16, tag="qt")
                nc.sync.dma_start(out=qt[:], in_=qq[:, :])
                osb = iop.tile([128, 2 * ROWS], f32, tag="osb")

                kvt = None
                for r in range(ROWS):
                    g, m = divmod(r, RPG)
                    if m == 0:
                        kvt = kvp.tile([128, GW], bf16, tag="kv")
                        nc.sync.dma_start(out=kvt[:], in_=kv[g])
                    base = m * 2 * N

                    # scores_T[t, c] = sum_d K[d, c*128+t] * q[d]
                    sc = pp_sc.tile([128, NCH], f32, tag="sc")
                    for c in range(NCH):
                        nc.tensor.matmul(
                            sc[:, c:c + 1],
                            kvt[:, base + 128 * c:base + 128 * (c + 1)],
                            qt[:, r:r + 1],
                            start=True, stop=True,
                        )
                    ssb = sp.tile([128, NCH], f32, tag="ssb")
                    nc.vector.tensor_add(ssb[:], sc[:], mtt[:, r * NCH:(r + 1) * NCH])
                    p = sp.tile([128, NCH], bf16, tag="p")
                    nc.scalar.activation(
                        p[:], ssb[:], mybir.ActivationFunctionType.Exp,
                        scale=SM_SCALE, accum_out=osb[:, ROWS + r:ROWS + r + 1],
                    )

                    # out[d] = sum_t P_T[t] * V_T[t, d], V chunk stationary
                    ov = pp_ov.tile([128, 1], f32, tag="ov")
                    vbase = base + N
                    for c in range(NCH):
                        nc.tensor.matmul(
                            ov[:],
                            kvt[:, vbase + 128 * c:vbase + 128 * (c + 1)],
                            p[:, c:c + 1],
                            start=(c == 0), stop=(c == NCH - 1),
                        )
                    nc.vector.tensor_copy(osb[:, r:r + 1], ov[:])

                nc.sync.dma_start(out=out[:, :], in_=osb[:])
    nc.compile()
    return nc


_NC_CACHE = None
_LAST_RES = None
_LAST_IN_MAPS = None


def kernel(q, k_cache, v_cache, block_tables, context_lens, layout_crow, layout_col):
    import ml_dtypes
    from concourse.bass_utils import run_bass_kernel_spmd

    bf16 = ml_dtypes.bfloat16
    q = np.asarray(q, np.float32)
    k_cache = np.asarray(k_cache, np.float32)
    v_cache = np.asarray(v_cache, np.float32)
    block_tables = np.asarray(block_tables, np.int32)
    context_lens = np.asarray(context_lens, np.int32)
    layout_crow = np.asarray(layout_crow, np.int32)
    layout_col = np.asarray(layout_col, np.int32)

    # ---- host: resolve CSR rows into gathered K/V panels (mirrors reference) ----
    q_pid = context_lens.astype(np.int64) - 1            # [B]
    pbid = q_pid // BLK
    h_idx = np.arange(H)
    hkv = h_idx // (H // KVH)
    start = layout_crow[h_idx[None, :], pbid[:, None]]   # [B,H]
    end = layout_crow[h_idx[None, :], pbid[:, None] + 1]
    jj = np.arange(J)
    idx = start[:, :, None] + jj                         # [B,H,J]
    valid = idx < end[:, :, None]
    idx = np.clip(idx, 0, layout_col.shape[1] - 1)
    cols = layout_col[h_idx[None, :, None], idx]         # [B,H,J]
    bt = block_tables[np.arange(B)[:, None, None], cols]  # [B,H,J]

    kcb = k_cache.astype(bf16)
    vcb = v_cache.astype(bf16)
    kb = kcb[bt, hkv[None, :, None]]                     # [B,H,J,32,16,4]
    # K d-major: Kd[b,h,d,(j,n)] with d = dx*4+xi
    kd_full = np.ascontiguousarray(
        kb.transpose(0, 1, 3, 5, 2, 4).reshape(B, H, 128, N)
    )
    vb = vcb[bt, hkv[None, :, None]]                     # [B,H,J,128,16]
    # V token-major, chunk-interleaved: Vt[b,h,t,(c,d)] = V[d, c*128+t]
    v_t = vb.transpose(0, 1, 2, 4, 3).reshape(B, H, N, 128)          # [(j,n), d]
    vt_full = np.ascontiguousarray(
        v_t.reshape(B, H, NCH, 128, 128).transpose(0, 1, 3, 2, 4)
        .reshape(B, H, 128, N)
    )
    pos = cols[..., None] * BLK + np.arange(BLK)          # [B,H,J,BLK]
    mask = valid[..., None] & (pos <= q_pid[:, None, None, None])
    mask = mask.reshape(B, H, N)
    madd = np.where(mask, np.float32(0.0), np.float32(-1e9))
    # mask_T[t, r*NCH + c] = madd[r, c*128 + t]
    qbf = q.astype(bf16)

    global _NC_CACHE, _LAST_RES
    if _NC_CACHE is None:
        _NC_CACHE = _build_device_program()
    nc = _NC_CACHE

    in_maps = []
    for core in range(NC_CORES):
        bs = slice(core * SEQ_PER_CORE, (core + 1) * SEQ_PER_CORE)
        kd_c = kd_full[bs].reshape(ROWS, 128, N)
        vt_c = vt_full[bs].reshape(ROWS, 128, N)
        kvg = np.empty((NG, 128, RPG, 2, N), bf16)
        kvg[:, :, :, 0, :] = kd_c.reshape(NG, RPG, 128, N).transpose(0, 2, 1, 3)
        kvg[:, :, :, 1, :] = vt_c.reshape(NG, RPG, 128, N).transpose(0, 2, 1, 3)
        mt_c = np.ascontiguousarray(
            madd[bs].reshape(ROWS, NCH, 128).transpose(2, 0, 1)
            .reshape(128, ROWS * NCH)
        )
        q_c = np.ascontiguousarray(qbf[bs].reshape(ROWS, 128).T)
        in_maps.append({
            "kv": kvg.reshape(NG, 128, GW),
            "mt": mt_c,
            "qq": q_c,
        })

    global _LAST_IN_MAPS
    _LAST_IN_MAPS = in_maps
    res = run_bass_kernel_spmd(nc, in_maps, core_ids=list(range(NC_CORES)))
    _LAST_RES = res
    out = np.empty((B, H, D), np.float32)
    for core in range(NC_CORES):
        o = res.results[core]["out"]                     # [128, 2*ROWS] fp32
        ov = o[:, :ROWS]
        denom = o[:, ROWS:].sum(axis=0)                  # [ROWS]
        bs = slice(core * SEQ_PER_CORE, (core + 1) * SEQ_PER_CORE)
        out[bs] = (ov / denom[None, :]).T.reshape(SEQ_PER_CORE, H, D)
    return out


# revision 7
# speedup vs baseline: 146.6117x; 2.2753x over previous
"""Local-strided block-sparse paged attention (decode) on 8 Trainium2 cores.

Sharding: the 64 (sequence, kv-head) pairs are bin-packed across 8 cores x
8 slots. For each pair, the 4 q-heads of the kv-head group share one
deduplicated K/V panel (union of the 4 heads' CSR rows), so each K/V block
is streamed from HBM once instead of up to 4 times. Panels are variable
length; slot k has the same chunk count on every core (max over the 8 pairs
assigned to that slot), keeping the program SPMD. The program is built per
slot-size signature and cached.

Host (numpy) resolves CSR + block_tables into bf16 gathered panels and
additive masks; the device does QK -> exp -> PV with per-head masks and
ships back the unnormalized PV output plus per-partition exp-sums; the
final normalization (a [128]-sum and divide per row) happens on host.
"""
import numpy as np

B, H, KVH, D, X = 16, 16, 4, 128, 4
HPG = H // KVH              # q-heads per kv-head group (4)
BLK, MAXB = 16, 256
NC_CORES = 8
NSLOT = 8                   # (seq, kv-group) pairs per core
SM_SCALE = 1.0 / float(np.sqrt(D))


def _build_device_program(slot_nc, reps=1):
    """slot_nc: tuple of per-slot chunk counts (shared across cores)."""
    import concourse.bacc as bacc
    import concourse.mybir as mybir
    from concourse.tile import TileContext
    import contextlib

    f32 = mybir.dt.float32
    bf16 = mybir.dt.bfloat16
    tot = sum(slot_nc)
    W = 2 * 128 * tot           # bf16 elems per partition line of kv panel
    MW = HPG * tot              # mask columns
    RC = NSLOT * HPG            # result columns (32)

    nc = bacc.Bacc("TRN2", target_bir_lowering=False)
    kv = nc.dram_tensor("kv", [128, W], bf16, kind="ExternalInput")
    mt = nc.dram_tensor("mt", [128, MW], f32, kind="ExternalInput")
    qq = nc.dram_tensor("qq", [128, RC], bf16, kind="ExternalInput")
    out = nc.dram_tensor("out", [128, 2 * RC], f32, kind="ExternalOutput")

    with TileContext(nc) as tc:
        with (
            tc.tile_pool(name="kv", bufs=1) as kvp,
            tc.tile_pool(name="small", bufs=4) as sp,
            tc.tile_pool(name="ps_sc", bufs=3, space="PSUM") as pp_sc,
            tc.tile_pool(name="ps_ov", bufs=3, space="PSUM") as pp_ov,
            tc.tile_pool(name="io", bufs=1) as iop,
        ):
            rep_ctx = tc.For_i(0, reps, 1) if reps > 1 else contextlib.nullcontext()
            with rep_ctx:
                mtt = iop.tile([128, MW], f32, tag="mt")
                nc.sync.dma_start(out=mtt[:], in_=mt[:, :])
                qt = iop.tile([128, RC], bf16, tag="qt")
                nc.sync.dma_start(out=qt[:], in_=qq[:, :])
                osb = iop.tile([128, 2 * RC], f32, tag="osb")

                woff = moff = 0
                for k in range(NSLOT):
                    ncK = slot_nc[k]
                    kw = 2 * 128 * ncK
                    kvt = kvp.tile([128, kw], bf16, tag=f"kv{k}")
                    nc.sync.dma_start(out=kvt[:], in_=kv[:, woff:woff + kw])

                    # scores_T[t, c*4+hh] = sum_d K[d, c*128+t] * q_hh[d]
                    sc = pp_sc.tile([128, HPG * ncK], f32, tag="sc")
                    for c in range(ncK):
                        nc.tensor.matmul(
                            sc[:, HPG * c:HPG * (c + 1)],
                            kvt[:, 128 * c:128 * (c + 1)],
                            qt[:, HPG * k:HPG * (k + 1)],
                            start=True, stop=True,
                        )
                    ssb = sp.tile([128, HPG * ncK], f32, tag="ssb")
                    nc.vector.tensor_add(
                        ssb[:], sc[:], mtt[:, HPG * moff:HPG * (moff + ncK)])
                    p = sp.tile([128, HPG * ncK], bf16, tag="p")
                    nc.scalar.activation(
                        p[:], ssb[:], mybir.ActivationFunctionType.Exp,
                        scale=SM_SCALE,
                    )
                    for hh in range(HPG):
                        nc.vector.reduce_sum(
                            osb[:, RC + HPG * k + hh:RC + HPG * k + hh + 1],
                            p[:, hh::HPG],
                            axis=mybir.AxisListType.X,
                        )

                    # out[d, hh] = sum_t P_T[t, hh] * V_T[t, d]
                    ov = pp_ov.tile([128, HPG], f32, tag="ov")
                    vbase = 128 * ncK
                    for c in range(ncK):
                        nc.tensor.matmul(
                            ov[:],
                            kvt[:, vbase + 128 * c:vbase + 128 * (c + 1)],
                            p[:, HPG * c:HPG * (c + 1)],
                            start=(c == 0), stop=(c == ncK - 1),
                        )
                    nc.vector.tensor_copy(osb[:, HPG * k:HPG * (k + 1)], ov[:])
                    woff += kw
                    moff += ncK

                nc.sync.dma_start(out=out[:, :], in_=osb[:])
    nc.compile()
    return nc


_NC_CACHE = {}
_LAST_RES = None
_LAST_IN_MAPS = None
_LAST_SLOT_NC = None


def kernel(q, k_cache, v_cache, block_tables, context_lens, layout_crow, layout_col):
    import ml_dtypes
    from concourse.bass_utils import run_bass_kernel_spmd

    bf16 = ml_dtypes.bfloat16
    q = np.asarray(q, np.float32)
    k_cache = np.asarray(k_cache, np.float32)
    v_cache = np.asarray(v_cache, np.float32)
    block_tables = np.asarray(block_tables, np.int32)
    context_lens = np.asarray(context_lens, np.int32)
    layout_crow = np.asarray(layout_crow, np.int32)
    layout_col = np.asarray(layout_col, np.int32)

    q_pid = context_lens.astype(np.int64) - 1            # [B]
    pbid = q_pid // BLK

    # ---- plan: per (b,g) dedup union + sizes ----
    pairs = []                                           # (b, g, U, cols_per_head)
    sizes = np.empty(B * KVH, np.int64)
    for b in range(B):
        for g in range(KVH):
            cols_h = []
            for hh in range(HPG):
                h = HPG * g + hh
                s, e = layout_crow[h, pbid[b]], layout_crow[h, pbid[b] + 1]
                cols_h.append(layout_col[h, s:e])
            U = np.unique(np.concatenate(cols_h))
            nC = max(1, -(-(len(U) * BLK) // 128))
            sizes[len(pairs)] = nC
            pairs.append((b, g, U, cols_h))

    # bin-pack: rank pairs by size desc; slot k gets ranks [8k, 8k+8),
    # one per core; slot size = max of the group (= first of the group)
    order = np.argsort(-sizes, kind="stable")
    slot_nc = tuple(int(sizes[order[NC_CORES * k]]) for k in range(NSLOT))
    assign = [[int(order[NC_CORES * k + j]) for k in range(NSLOT)]
              for j in range(NC_CORES)]                  # [core][slot] -> pair idx

    kcb = k_cache.astype(bf16)
    vcb = v_cache.astype(bf16)
    tok16 = np.arange(BLK, dtype=np.int64)

    in_maps = []
    for core in range(NC_CORES):
        kv_parts, m_parts, q_cols = [], [], []
        for k in range(NSLOT):
            b, g, U, cols_h = pairs[assign[core][k]]
            ncK = slot_nc[k]
            nB, nT = ncK * (128 // BLK), ncK * 128
            nU = len(U)
            U_pad = np.full(nB, U[0], np.int64)
            U_pad[:nU] = U
            btp = block_tables[b, U_pad].astype(np.int64)  # physical block ids

            kp = kcb[btp, g]                             # [nB,32,16,4]
            kp = kp.transpose(1, 3, 0, 2).reshape(128, nT)
            vp = vcb[btp, g]                             # [nB,128,16]
            vp = (vp.transpose(0, 2, 1).reshape(ncK, 128, 128)
                  .transpose(1, 0, 2).reshape(128, nT))
            kv_parts += [kp, vp]

            real = np.arange(nB) < nU                    # [nB]
            pos_ok = (U_pad[:, None] * BLK + tok16[None, :]) <= q_pid[b]
            mask4 = np.empty((nB, BLK, HPG), bool)
            for hh in range(HPG):
                member = np.isin(U_pad, cols_h[hh]) & real
                mask4[:, :, hh] = member[:, None] & pos_ok
            madd = np.where(mask4, np.float32(0.0), np.float32(-1e9))
            # [nB,16,4] -> [nT,4] -> [nC,128,4] -> [128, nC*4]
            m_parts.append(
                madd.reshape(nT, HPG).reshape(ncK, 128, HPG)
                .transpose(1, 0, 2).reshape(128, ncK * HPG))
            q_cols.append(q[b, HPG * g:HPG * (g + 1)].T)   # [128, 4]

        in_maps.append({
            "kv": np.ascontiguousarray(np.concatenate(kv_parts, axis=1)),
            "mt": np.ascontiguousarray(np.concatenate(m_parts, axis=1)),
            "qq": np.ascontiguousarray(
                np.concatenate(q_cols, axis=1)).astype(bf16),
        })

    global _LAST_RES, _LAST_IN_MAPS, _LAST_SLOT_NC
    if slot_nc not in _NC_CACHE:
        _NC_CACHE[slot_nc] = _build_device_program(slot_nc)
    nc = _NC_CACHE[slot_nc]
    _LAST_IN_MAPS = in_maps
    _LAST_SLOT_NC = slot_nc

    res = run_bass_kernel_spmd(nc, in_maps, core_ids=list(range(NC_CORES)))
    _LAST_RES = res
    RC = NSLOT * HPG
    out = np.empty((B, H, D), np.float32)
    for core in range(NC_CORES):
        o = res.results[core]["out"]                     # [128, 2*RC] fp32
        denom = o[:, RC:].sum(axis=0)                    # [RC]
        for k in range(NSLOT):
            b, g, _, _ = pairs[assign[core][k]]
            cols = slice(HPG * k, HPG * (k + 1))
            out[b, HPG * g:HPG * (g + 1)] = (
                o[:, cols] / denom[cols][None, :]).T
    return out
